# revision 1
# baseline (speedup 1.0000x reference)
"""Trainium2 Bass kernel for DifferentiableVietorisRips.

Output M = concat([eye(N); pair-masks; triple-masks]) with
  N = 128, D = 512, EPSILON = 32.0, SHARPNESS = 10.0, VR_DIM = 2
  pair rows   : P = C(128,2) = 8128,  row(i,j) has sigmoid(10*(32-d_ij)) at cols i,j
  triple rows : T = C(128,3) = 341376, row(i,j,k) has [d_ij<=32 & d_jk<=32 & d_ik<=32]
                at cols i,j,k
  M shape [349632, 128] float32.

Sharding: contiguous row chunks, 43704 rows/core across 8 cores. One uniform
SPMD Bass program; every per-core difference lives in input tensors.

Per-core device pipeline (CoreSim ~111us vs ~313us for the first working
version; DMA output bandwidth, the two table-gathers, and the bv-table
broadcast are the remaining cost centers):
  1. dist [128,128] via PE: d2 = sum_k(-2 W^T)^T W^T + (sq x 1 + 1 x sq), then
     ACT sqrt(max(0, d2)).
  2. Packed value table BV = 2*(dist<=eps) + sigmoid(10*(eps-dist)), bf16,
     flattened through a DRAM staging row onto partitions {0,32,64} and
     replicated to every partition ([128, 16384] f32) via K=1 PE
     ones-broadcasts; PSUM drained by alternating DVE/ACT copies.
  3. TWO GPSIMD ap_gathers fetch BV at d(i,j) and at interleaved
     d(j,k)/d(i,k) per output row (static per-core indices, 16x
     group-redundant; ap_gather cost is max(table elems, output slots), so
     the merged second call amortizes one whole table scan).  Decode to
     {0,1} bf16 fuses the triple-AND via strided (block, t, residue) views,
     compacts via static residue mask + reduce -> cond per row.  The d(i,j)
     gather doubles as the pair-sigmoid source: sigma = BV - 2*(BV>=1.5).
     The last decode and its compaction are column-split so the mask
     phase starts before half B finishes decoding.
  4. Per-row scale table s [128, NB]: cond on triple rows, sigma on pair
     rows, 1 on eye rows, 0 on padding.
  5. Mask supertiles (16 blocks of 128 rows), two producers in parallel:
     - supertiles 0..11 (eye/pair + first triples): one bf16 TensorScalar
       multiply per block, s[:, b] x PATTERN block, with the static 0/1
       pattern streamed from DRAM as fp8 via casting SWDGE DMAs;
     - supertiles 12..21 (all-triple): GPSIMD local_scatter writes s at the
       three static one-hot columns per row (no pattern traffic at all).
  6. Supertile -> one batched HWDGE DMA (alternating SP/ACT rings) into a
     partition-major bf16 DRAM shard; the host undoes the block interleave
     and casts to f32 (values are exact/bf16-rounded already).
"""

import numpy as np
import ml_dtypes
from contextlib import ExitStack

import concourse.bacc as bacc
import concourse.tile as tile
from concourse import library_config, mybir
from concourse.bass_utils import run_bass_kernel_spmd
from concourse.tile_rust import add_dep_helper

# ---------------------------------------------------------------- constants
N = 128
D = 512
EPS = 32.0
SHARP = 10.0
NCORES = 8

P_PAIRS = N * (N - 1) // 2            # 8128
T_TRIS = N * (N - 1) * (N - 2) // 6   # 341376
R_TOT = N + P_PAIRS + T_TRIS          # 349632
RC = R_TOT // NCORES                  # 43704 rows per core
NB = (RC + 127) // 128                # 342 blocks per core (last has 56 rows)
NBV = 65                              # blocks holding eye/pair rows (core 0)
SUPER = 16                            # blocks per supertile / output DMA
NSUP = (NB + SUPER - 1) // SUPER      # 22 (last has 6 blocks incl. tail)
CHA = 11 * SUPER                      # decode half A: blocks 0..175
SSC = 12                              # first Pool-scattered supertile
BSC = SSC * SUPER                     # first scattered block (all-triple)

_DT = mybir.dt


# ---------------------------------------------------------------- host tables
def _host_tables():
    """Static per-core tensors (independent of W)."""
    iu, ju = np.triu_indices(N, k=1)                      # pair lex order
    ti, tj, tk = [], [], []
    for i in range(N - 2):
        for j in range(i + 1, N - 1):
            ks = np.arange(j + 1, N)
            ti.append(np.full(len(ks), i))
            tj.append(np.full(len(ks), j))
            tk.append(ks)
    ti = np.concatenate(ti).astype(np.int64)
    tj = np.concatenate(tj).astype(np.int64)
    tk = np.concatenate(tk).astype(np.int64)
    assert ti.shape[0] == T_TRIS

    # global row r -> col indices (-1 = unused), selector classes, gather idx
    c1 = np.full(R_TOT, -1, np.int64)  # first one-hot col (i or eye col)
    c2 = np.full(R_TOT, -1, np.int64)  # second (j)
    c3 = np.full(R_TOT, -1, np.int64)  # third (k)
    cm = np.zeros(R_TOT, np.float32)   # 1 iff triple row
    vs = np.zeros(R_TOT, np.float32)   # 1 iff pair row
    m0 = np.zeros(R_TOT, np.float32)   # 1 iff eye row
    ix1 = np.zeros(R_TOT, np.int16)    # flat idx into dist for d(i,j) (+pairs)
    ix2 = np.zeros(R_TOT, np.int16)    # d(j,k)
    ix3 = np.zeros(R_TOT, np.int16)    # d(i,k)

    r = np.arange(N)
    c1[:N] = r
    m0[:N] = 1.0

    s = N
    c1[s:s + P_PAIRS] = iu
    c2[s:s + P_PAIRS] = ju
    vs[s:s + P_PAIRS] = 1.0
    ix1[s:s + P_PAIRS] = (iu * N + ju).astype(np.int16)

    s = N + P_PAIRS
    c1[s:] = ti
    c2[s:] = tj
    c3[s:] = tk
    cm[s:] = 1.0
    ix1[s:] = (ti * N + tj).astype(np.int16)
    ix2[s:] = (tj * N + tk).astype(np.int16)
    ix3[s:] = (ti * N + tk).astype(np.int16)

    def shard(a, core, nb):
        """rows [core*RC, core*RC+128*nb) -> [128, nb] (p, b), zero padded."""
        lo = core * RC
        take = min(RC, 128 * nb, a.shape[0] - lo)
        full = np.zeros(128 * nb, a.dtype)
        full[:take] = a[lo:lo + take]
        return full.reshape(nb, 128).T.copy()

    def pattern(core):
        """[128, NB*128] bf16: PATTERN[p, 128*b + c] for global row 128*b+p."""
        lo = core * RC
        pat = np.zeros((NB * 128, 128), np.float32)
        rr = np.arange(RC)
        for cols in (c1, c2, c3):
            cc = cols[lo:lo + RC]
            ok = cc >= 0
            pat[rr[ok], cc[ok]] = 1.0
        pat = pat.reshape(NB, 128, 128).transpose(1, 0, 2).reshape(128, NB * 128)
        return pat.astype(ml_dtypes.float8_e4m3)

    per_core = []
    for c in range(NCORES):
        per_core.append({
            "PAT": pattern(c),
            "CM": shard(cm, c, NB).astype(ml_dtypes.bfloat16),
            "VSEL": shard(vs, c, NBV).astype(ml_dtypes.bfloat16),
            "M0S": shard(m0, c, NBV).astype(ml_dtypes.bfloat16),
            "IXALL": np.stack(
                [shard(ix1, c, NB), shard(ix2, c, NB), shard(ix3, c, NB)],
                axis=2,
            ).reshape(128, 3 * NB),
        })

    def scatter_idx(core):
        """[128, 3*(NB-BSC)] i16: scatter cols (local to 8-block groups) for
        blocks BSC..NB, -1 where the row has no such one-hot col."""
        lo = core * RC
        nsc = NB - BSC
        sidx = np.full((128, 3 * nsc), -1, np.int16)
        for t, cols in enumerate((c1, c2, c3)):
            cs = np.full(NB * 128, -1, np.int64)
            take = min(RC, c1.shape[0] - lo)
            cs[:take] = cols[lo:lo + take]
            cs = cs.reshape(NB, 128).T  # [p, b]
            for b in range(BSC, NB):
                v = cs[:, b]
                loc = 128 * ((b - BSC) % 8) + v
                sidx[:, 3 * (b - BSC) + t] = np.where(v >= 0, loc, -1)
        return sidx

    for c in range(NCORES):
        per_core[c]["SIDX"] = scatter_idx(c)

    ident = np.eye(128, dtype=np.float32)
    # residue mask for gather compaction: m16[p, 16*b + r] = (p % 16 == r)
    rmod = (np.arange(128) % 16)[:, None]
    rr = np.tile(np.arange(16), NB)[None, :]
    m16 = (rmod == rr).astype(ml_dtypes.bfloat16)
    return per_core, ident, m16


# ---------------------------------------------------------------- bass program
def _build_program():
    # Bacc (not raw Bass): lowers Tile's multi-wait drain/barrier sync into
    # walrus-encodable form and auto-inserts modify_pool_config for
    # load_library. detect_race_conditions=False: the sim's race shadow
    # mis-models some APs; ordering is via Tile deps + add_dep_helper edges.
    nc = bacc.Bacc(
        "TRN2", target_bir_lowering=False, debug=False,
        detect_race_conditions=False,
    )

    f32, bf16, u16 = _DT.float32, _DT.bfloat16, _DT.int16
    fp8 = _DT.float8e4
    W_p = nc.declare_dram_parameter("W", [N, D], f32, isOutput=False)
    # host-marshalled transposes of the runtime input W: kills 4 PE
    # transposes + 8 DVE ops off the critical prefix
    WT_p = nc.declare_dram_parameter("WT", [N, D], f32, isOutput=False)
    WTM2_p = nc.declare_dram_parameter("WTM2", [N, D], f32, isOutput=False)
    IDENT_p = nc.declare_dram_parameter("IDENT", [128, 128], f32, isOutput=False)
    PAT_p = nc.declare_dram_parameter("PAT", [128, NB * 128], fp8, isOutput=False)
    # DRAM staging row for the bv flatten (descriptor-friendly two-hop)
    # 512-elem pad: the single strided flatten load reads 3x5632 elems
    BVD_p = nc.declare_dram_parameter("bvd", [3 * 5632], bf16, isOutput=True)
    CM_p = nc.declare_dram_parameter("CM", [128, NB], bf16, isOutput=False)
    VSEL_p = nc.declare_dram_parameter("VSEL", [128, NBV], bf16, isOutput=False)
    M0S_p = nc.declare_dram_parameter("M0S", [128, NBV], bf16, isOutput=False)
    IXALL_p = nc.declare_dram_parameter("IXALL", [128, 3 * NB], u16, isOutput=False)
    M16_p = nc.declare_dram_parameter("M16", [128, 16 * NB], bf16, isOutput=False)
    SIDX_p = nc.declare_dram_parameter(
        "SIDX", [128, 3 * (NB - BSC)], u16, isOutput=False
    )
    # partition-major output: out[p, 128*b + c] = M[128*b + p, c].  4KB
    # contiguous per-partition runs keep the DMA at full descriptor size; the
    # host undoes the block interleave (cheap numpy transpose).
    OUT_p = nc.declare_dram_parameter("out", [128, NB * 128], bf16, isOutput=True)

    with tile.TileContext(nc) as tc, ExitStack() as ctx:
        const = ctx.enter_context(tc.tile_pool(name="const", bufs=1))
        work = ctx.enter_context(tc.tile_pool(name="work", bufs=1))
        psum = ctx.enter_context(tc.tile_pool(name="psum", bufs=1, space="PSUM"))
        psum2 = ctx.enter_context(tc.tile_pool(name="psum2", bufs=1, space="PSUM"))
        psumb = ctx.enter_context(tc.tile_pool(name="psumb", bufs=4, space="PSUM"))
        gpool = ctx.enter_context(tc.tile_pool(name="gath", bufs=1))
        pat = ctx.enter_context(tc.tile_pool(name="pat", bufs=3))
        sup = ctx.enter_context(tc.tile_pool(name="sup", bufs=2))

        # first Pool-engine instruction: select the ucode library that
        # implements InstAPGather (the only custom gpsimd op we use)
        nc.gpsimd.load_library(library_config.ap_gather)

        load_instrs = {}

        def load(pool, param, shape, dt):
            t = pool.tile(shape, dt, tag=param.name)
            load_instrs[param.name] = nc.sync.dma_start(t[:], param.ap())
            return t

        w_sb = load(const, W_p, [N, D], f32)
        wt_sb = load(const, WT_p, [N, D], f32)
        wtm2_sb = load(const, WTM2_p, [N, D], f32)
        ident = load(const, IDENT_p, [128, 128], f32)
        cmt = load(const, CM_p, [128, NB], bf16)
        vselt = load(const, VSEL_p, [128, NBV], bf16)
        m0st = load(const, M0S_p, [128, NBV], bf16)
        ixall = load(const, IXALL_p, [128, 3 * NB], u16)
        sidx = load(const, SIDX_p, [128, 3 * (NB - BSC)], u16)
        # zero the bvd pad tail early, off the flatten critical path

        # ---- 1. dist --------------------------------------------------------
        gall = gpool.tile([128, 16 * 3 * NB], f32, tag="gall")
        # dist-chain scratch aliases the gather buffer: every use below is
        # ordered before the gather's (dep-chained) write of gall
        ww = gall[:, 0:D]
        nc.vector.tensor_tensor(ww, w_sb[:], w_sb[:], mybir.AluOpType.mult)
        sq = work.tile([N, 1], f32)
        nc.vector.tensor_reduce(
            sq[:], ww, mybir.AxisListType.X, mybir.AluOpType.add
        )


        # aug_l = [sq_row; ones], aug_r = [ones; sq_row] via PE transpose of
        # [128, 2] column pairs (engines can't write at partition offset 1)
        cat_l = work.tile([128, 2], f32)
        nc.vector.tensor_copy(cat_l[:, 0:1], sq[:])
        nc.vector.memset(cat_l[:, 1:2], 1.0)
        cat_r = work.tile([128, 2], f32)
        nc.vector.memset(cat_r[:, 0:1], 1.0)
        nc.vector.tensor_copy(cat_r[:, 1:2], sq[:])
        paug_l = psum2.tile([2, 128], f32, tag="paug")
        nc.tensor.transpose(paug_l[:], cat_l[:], ident[:])
        aug_l = work.tile([2, 128], f32)
        nc.vector.tensor_copy(aug_l[:], paug_l[:])
        paug_r = psum2.tile([2, 128], f32, tag="paug")
        nc.tensor.transpose(paug_r[:], cat_r[:], ident[:])
        aug_r = work.tile([2, 128], f32)
        nc.vector.tensor_copy(aug_r[:], paug_r[:])

        d2 = psum.tile([128, 128], f32, tag="d2")
        for c4 in range(4):
            sl4 = slice(c4 * 128, (c4 + 1) * 128)
            nc.tensor.matmul(
                d2[:], wtm2_sb[:, sl4], wt_sb[:, sl4], start=(c4 == 0), stop=False
            )
        nc.tensor.matmul(d2[:], aug_l[:], aug_r[:], start=False, stop=True)

        dmax = gall[:, D:D + 128]
        nc.vector.tensor_scalar_max(dmax, d2[:], 0.0)
        dist = gall[:, D + 128:D + 256]
        nc.scalar.activation(dist, dmax, mybir.ActivationFunctionType.Sqrt)

        # ---- 2. packed table: BV = 2*(dist<=eps) + sigmoid(10*(eps-dist)) ---
        # one f32 table serves both the triple condition (BV >= 1.5) and the
        # pair value (BV - 2*(BV >= 1.5)); sigmoid in (0,1) keeps the bands
        # [0,1) and [2,3) cleanly separable.
        bind = gall[:, D + 256:D + 384]
        nc.vector.tensor_scalar(bind, dist, EPS, None, mybir.AluOpType.is_le)
        sigb = work.tile([128, 1], f32)
        nc.vector.memset(sigb[:], SHARP * EPS)
        sgm = gall[:, D + 384:D + 512]
        nc.scalar.activation(
            sgm, dist, mybir.ActivationFunctionType.Sigmoid,
            bias=sigb[:], scale=-SHARP,
        )
        bv = gall[:, D + 512:D + 640]
        nc.vector.scalar_tensor_tensor(
            bv, bind, 2.0, sgm,
            mybir.AluOpType.mult, mybir.AluOpType.add,
        )
        bvb = work.tile([128, 128], bf16)
        nc.vector.tensor_copy(bvb[:], bv)

        # replicate bv (flattened) to every partition: bvtab[p, 128*i + c] =
        # bv[i, c].  Flatten bf16 onto partitions {0,32,64} (PE operands must
        # sit at one of those base partitions; stationary and moving share
        # it), cols [0:128) hold the ones row for the K=1 broadcasts.  The
        # flatten goes through a DRAM staging row: SBUF->DRAM keeps 256B
        # descriptors, DRAM->single-partition-SBUF is one big descriptor --
        # much cheaper than a direct cross-partition SBUF gather.  Tile does
        # not track deps through DRAM tensors; wire them explicitly.
        chunk_of = [min(ch // 11, 2) for ch in range(32)]  # 11/11/10 split
        bvflat = work.tile([128, 128 + 11 * 512], bf16)
        nc.vector.memset(bvflat[:, 0:128], 1.0)
        bvd_w = nc.sync.dma_start(BVD_p.ap()[0:128 * 128], bvb[:])
        flat_loads = []
        flat_engines = [nc.sync, nc.scalar, nc.gpsimd]  # parallel DGE paths
        for g in range(3):
            lo = chunk_of.index(g) * 4            # first bv row of group g
            hi = 32 * 4 if g == 2 else (chunk_of.index(g + 1)) * 4
            fl = flat_engines[g].dma_start(
                bvflat[32 * g:32 * g + 1, 128:128 + (hi - lo) * 128],
                BVD_p.ap()[lo * 128:hi * 128],
            )
            add_dep_helper(fl.ins, bvd_w.ins, reason="flatten RAW via DRAM")
            flat_loads.append(fl)
        # the 1.4MB M16 table is not needed until the first gather decode;
        # keep its transfer out of the latency-critical flatten window
        m16rep = load(const, M16_p, [128, 16 * NB], bf16)
        for fl2 in flat_loads:
            add_dep_helper(
                load_instrs["M16"].ins, fl2.ins, reason="M16 after flatten"
            )
        bvtab = work.tile([128, 128 * 128], f32)
        rep_writers = []
        for ch in range(32):
            g = chunk_of[ch]
            h = ch - chunk_of.index(g)
            sl = slice(ch * 512, (ch + 1) * 512)
            msl = slice(128 + h * 512, 128 + (h + 1) * 512)
            pbc = psumb.tile([128, 512], f32, tag="pbc")
            mm = nc.tensor.matmul(
                pbc[:], bvflat[32 * g:32 * g + 1, 0:128],
                bvflat[32 * g:32 * g + 1, msl], start=True, stop=True,
            )
            add_dep_helper(mm.ins, flat_loads[g].ins, reason="bcast after flat")
            if ch % 2 == 0:
                rep_writers.append(nc.vector.tensor_copy(bvtab[:, sl], pbc[:]))
            else:
                rep_writers.append(nc.scalar.copy(bvtab[:, sl], pbc[:]))

        # ---- 3. gathers -> cond + pair sigma --------------------------------
        # ap_gather cost scales with the TABLE size (16K elems), not the index
        # count, so three full-width gathers beat six half-width ones.
        #
        # ap_gather uses group-shared indices: the 16 partitions of a Q7 core
        # hold distinct index lists, and every gathered value lands replicated
        # across the group's 16 output partitions. Slot y=16b+r of the output
        # serves partition-class r for block b; compaction is (multiply by the
        # static residue mask) then (reduce over r).  ap_gather's for_isa APs
        # are invisible to Tile's dep tracker; wire ordering explicitly.
        ge, mul, add_ = (
            mybir.AluOpType.is_ge, mybir.AluOpType.mult, mybir.AluOpType.add
        )

        # prefetch the first pattern supertiles before any Pool-queue work:
        # their SWDGE dispatches are cheap, and the transfers land during the
        # gather phase.  The rest are dispatched after the gathers (Pool is
        # free then) inside the mask loop.
        PREF_SET = (5, 6, 7)  # first supertiles to drain (triple-only)
        pcs = {}
        pc_loads = []
        for s_i in PREF_SET:
            csl = slice(s_i * SUPER * 128, min(NB, (s_i + 1) * SUPER) * 128)
            pc = pat.tile([128, SUPER * 128], bf16, tag="pat")
            pc_loads.append(
                nc.gpsimd.dma_start(pc[:, :csl.stop - csl.start], PAT_p.ap()[:, csl])
            )
            pcs[s_i] = pc

        bt0 = gpool.tile([128, 16 * NB], bf16, tag="b0")
        spv = gpool.tile([128, 16 * NBV], bf16, tag="spv")

        def gather(gt, ixt, ixt_name, nix, waits):
            gi = nc.gpsimd.ap_gather(
                gt[:], bvtab[:], ixt[:],
                channels=128, num_elems=128 * 128, d=1, num_idxs=nix,
            )
            for w in rep_writers:
                add_dep_helper(gi.ins, w.ins, reason="gather after table")
            add_dep_helper(
                gi.ins, load_instrs[ixt_name].ins, reason="gather after idx load"
            )
            for w in waits:
                add_dep_helper(gi.ins, w.ins, reason="gbuf reuse WAR")
            return gi

        def dep(di, gi):
            add_dep_helper(di.ins, gi.ins, reason="decode after gather")
            return di

        # ALL THREE lookups per row ride ONE table scan: ap_gather cost is
        # max(table elems, output slots) and 16*3*NB (=16416) barely exceeds
        # the 16384-elem table.  Decode walks strided views of the
        # (block, t, residue) slot nesting.
        gi_all = gather(gall, ixall, "IXALL", 16 * 3 * NB, [])
        gv = gall[:].rearrange("p (b t r) -> p b t r", t=3, r=16)
        btv = bt0[:].rearrange("p (b r) -> p b r", r=16)
        d1 = dep(nc.vector.tensor_scalar(
            btv[:, :, :], gv[:, :, 0, :], 1.5, None, ge), gi_all)
        # pair sigma from the t=0 slots: sigma = g - 2*(g>=1.5)
        sp1 = dep(nc.vector.scalar_tensor_tensor(
            spv[:].rearrange("p (b r) -> p b r", r=16),
            btv[:, :NBV, :], -2.0, gv[:, :NBV, 0, :], mul, add_
        ), gi_all)
        nc.vector.tensor_tensor(bt0[:], bt0[:], m16rep[:], mul)

        def decode23(lo, hi, after=None):
            for t in (1, 2):
                di = dep(nc.vector.scalar_tensor_tensor(
                    btv[:, lo:hi, :], gv[:, lo:hi, t, :], 1.5,
                    btv[:, lo:hi, :], ge, mul), gi_all)
                if after is not None:
                    add_dep_helper(di.ins, after.ins, reason="decode order")

        # column-split: half A's decode/reduce/scale run first so the mask
        # supertiles start draining while half B still decodes.
        decode23(0, CHA)

        # ---- 4. per-row scale table s = cc*CM + dpair*VSEL + M0S ------------
        cc = work.tile([128, NB], bf16)
        dpair = work.tile([128, NBV], bf16)
        sv = work.tile([128, NB], f32)
        pv = work.tile([128, NBV], f32)

        def sv_half(lo, hi):
            # exact despite bf16: ≤1 of the 16 reduced slots is nonzero
            with nc.allow_low_precision(reason="one-hot residue reduction"):
                red = nc.vector.tensor_reduce(
                    cc[:, lo:hi],
                    bt0[:, 16 * lo:16 * hi].rearrange("p (b r) -> p b r", r=16),
                    mybir.AxisListType.X, mybir.AluOpType.add,
                )
            nc.vector.tensor_tensor(
                sv[:, lo:hi], cc[:, lo:hi], cmt[:, lo:hi], mul
            )
            if lo == 0:
                with nc.allow_low_precision(reason="one-hot residue reduction"):
                    nc.vector.tensor_tensor(
                        spv[:], spv[:], m16rep[:, :16 * NBV], mul
                    )
                    nc.vector.tensor_reduce(
                        dpair[:], spv[:].rearrange("p (b r) -> p b r", r=16),
                        mybir.AxisListType.X, mybir.AluOpType.add,
                    )
                nc.vector.tensor_tensor(pv[:], dpair[:], vselt[:], mul)
                nc.vector.tensor_tensor(pv[:], pv[:], m0st[:], add_)
                nc.vector.tensor_tensor(sv[:, :NBV], sv[:, :NBV], pv[:], add_)
            return red

        # ---- 5. supertiles: mask = s * PATTERN, one DMA per supertile -------
        def mask_super(s_i):
            b_lo = s_i * SUPER
            b_hi = min(NB, b_lo + SUPER)
            nblk = b_hi - b_lo
            csl = slice(b_lo * 128, b_hi * 128)
            if s_i in pcs:
                pc = pcs[s_i]
            else:
                pc = pat.tile([128, SUPER * 128], bf16, tag="pat")
                nc.gpsimd.dma_start(pc[:, :nblk * 128], PAT_p.ap()[:, csl])
            st = sup.tile([128, SUPER * 128], bf16, tag="super")
            for b in range(b_lo, b_hi):
                sl = slice((b - b_lo) * 128, (b - b_lo + 1) * 128)
                nc.vector.tensor_scalar(
                    st[:, sl], pc[:, sl], sv[:, b:b + 1], None, mul
                )
            # batched output DMA, same partition-major layout as SBUF;
            # alternate HWDGE rings (SP/ACT) so dispatch+completion pipelines
            eng = nc.sync if s_i % 2 == 0 else nc.scalar
            eng.dma_start(OUT_p.ap()[:, csl], st[:, :nblk * 128])

        red_a = sv_half(0, CHA)
        # half-B decode waits for half A's compaction so the first supertiles
        # drain while B decodes
        # supers 5..10 need only triple conds (cols >= NBV); they drain while
        # the pair-sigma chain still holds up supers 0..4
        for s_i in range(5, 11):
            mask_super(s_i)
        for s_i in range(0, 5):
            mask_super(s_i)
        decode23(CHA, NB, after=red_a)
        sv_half(CHA, NB)
        mask_super(11)

        # ---- 6. Pool-scattered supertiles -----------------------------------
        # The remaining (all-triple) supertiles skip the pattern entirely:
        # gpsimd local_scatter zeroes the tile and writes s at the three
        # static one-hot columns per row, in parallel with the DVE TS chain.
        # sdata = s replicated x3 along the free dim (strided bf16 copies).
        nsc = NB - BSC
        sdatab = work.tile([128, 3 * nsc], bf16)
        sd_view = sdatab[:].rearrange("p (b t) -> p b t", t=3)
        sd_ops = []
        for t in range(3):
            sd_ops.append(nc.vector.tensor_copy(
                sd_view[:, :, t:t + 1],
                sv[:, BSC:NB].rearrange("p (b o) -> p b o", o=1),
            ))
        ll2 = nc.gpsimd.load_library(library_config.local_scatter)
        add_dep_helper(ll2.ins, gi_all.ins, reason="lib switch after gather")
        stsc0 = gpool.tile([128, SUPER * 128], bf16, tag="stsc0")
        stsc1 = gpool.tile([128, SUPER * 128], bf16, tag="stsc1")
        sc_prev = {}
        for s_i in range(SSC, NSUP):
            b_lo = s_i * SUPER
            b_hi = min(NB, b_lo + SUPER)
            nblk = b_hi - b_lo
            csl = slice(b_lo * 128, b_hi * 128)
            st = stsc0 if s_i % 2 == 0 else stsc1
            scs = []
            for h in range((nblk + 7) // 8):
                hb = min(8, nblk - 8 * h)
                a0 = (b_lo - BSC) + 8 * h
                sc = nc.gpsimd.local_scatter(
                    st[:, h * 1024:h * 1024 + hb * 128],
                    sdatab[:, 3 * a0:3 * (a0 + hb)],
                    sidx[:, 3 * a0:3 * (a0 + hb)],
                    channels=128, num_elems=hb * 128, num_idxs=3 * hb,
                )
                add_dep_helper(sc.ins, ll2.ins, reason="scatter after lib")
                for o in sd_ops:
                    add_dep_helper(sc.ins, o.ins, reason="scatter after sdata")
                add_dep_helper(
                    sc.ins, load_instrs["SIDX"].ins, reason="scatter after idx"
                )
                slot = s_i % 2
                if slot in sc_prev:
                    add_dep_helper(
                        sc.ins, sc_prev[slot].ins, reason="scatter buf WAR"
                    )
                scs.append(sc)
            eng = nc.sync if s_i % 2 == 0 else nc.scalar
            dma = eng.dma_start(OUT_p.ap()[:, csl], st[:, :nblk * 128])
            for sc in scs:
                add_dep_helper(dma.ins, sc.ins, reason="dma after scatter")
            sc_prev[s_i % 2] = dma

    nc.compile()
    return nc


_PROGRAM = None
_TABLES = None


def _get_program():
    global _PROGRAM, _TABLES
    if _PROGRAM is None:
        _TABLES = _host_tables()
        _PROGRAM = _build_program()
    return _PROGRAM, _TABLES


def _feeds(core, W, per_core, ident, m16):
    t = per_core[core]
    wt = np.ascontiguousarray(
        W.T.reshape(4, 128, 128).transpose(1, 0, 2).reshape(128, 512)
    )
    return {
        "W": W, "WT": wt, "WTM2": np.ascontiguousarray(-2.0 * wt),
        "IDENT": ident, "M16": m16,
        "PAT": t["PAT"], "CM": t["CM"], "VSEL": t["VSEL"], "M0S": t["M0S"],
        "IXALL": t["IXALL"],
        "SIDX": t["SIDX"],
    }


def _unshard(out_pm: np.ndarray) -> np.ndarray:
    """[128, NB*128] partition-major bf16 -> [RC, 128] f32."""
    return (
        out_pm.astype(np.float32)
        .reshape(128, NB, 128)
        .transpose(1, 0, 2)
        .reshape(NB * 128, 128)[:RC]
    )


def kernel(W: np.ndarray) -> np.ndarray:
    nc, (per_core, ident, m16) = _get_program()
    W = np.ascontiguousarray(np.asarray(W, dtype=np.float32))
    in_maps = [_feeds(c, W, per_core, ident, m16) for c in range(NCORES)]
    res = run_bass_kernel_spmd(nc, in_maps, list(range(NCORES)))
    shards = [_unshard(np.asarray(res.results[c]["out"])) for c in range(NCORES)]
    return np.concatenate(shards, axis=0)



# revision 24
# speedup vs baseline: 1.7125x; 1.7125x over previous
"""Trainium2 Bass kernel for DifferentiableVietorisRips.

Output M = concat([eye(N); pair-masks; triple-masks]) with
  N = 128, D = 512, EPSILON = 32.0, SHARPNESS = 10.0, VR_DIM = 2
  pair rows   : P = C(128,2) = 8128,  row(i,j) has sigmoid(10*(32-d_ij)) at cols i,j
  triple rows : T = C(128,3) = 341376, row(i,j,k) has [d_ij<=32 & d_jk<=32 & d_ik<=32]
                at cols i,j,k
  M shape [349632, 128] float32.

Sharding: contiguous row chunks, 43704 rows/core across 8 cores. One uniform
SPMD Bass program; every per-core difference lives in input tensors.

v3 pipeline (CoreSim: 103.9us baseline -> ~56us target):
  1. d2 [128,128] via PE (f32).  No sqrt: the adjacency bit compares in the
     squared domain (d2 <= eps^2) and sigma = sigmoid(SHARP/(2 eps) *
     (eps^2 - d2)) matches sigmoid(SHARP*(eps-d)) to ~7e-4 absolute (first-
     order at the threshold, saturated elsewhere).  Only ONE activation
     table (sigmoid set), preloaded by a dummy op at t~0, so no
     LoadActFuncSet sits on the critical path.
  2. Band table g = BAND*(d2<=eps^2) + sigma, with g[32,0] (a lower-triangle
     slot no row ever references) memset to 0: pair/eye rows point their
     2nd/3rd gather index there, so their residue sum is g0 alone and the
     pair sigma decode suffers no +2*(BAND+1) bf16 cancellation.
  3. bv -> DRAM row, then three partition-broadcast DMAs (SP/Act/Pool,
     stride-0 DRAM source) replicate it to bvtab [128,16384] f32 (~8.4us of
     overlapped DMA; no flatten, no K=1 matmuls, no PSUM drain).
  4. One merged ap_gather (16416 slots, ~13.7us Pool) fetches g(i,j),
     g(j,k), g(i,k) per output row in (block, t, residue) slot nesting.
  5. Decode split Pool/DVE by measured cost-model modes (TT f32 runs 1x on
     DVE but Pool TT is 0.83ns/elem; TT bf16 and TS run 0.5x on DVE):
     Pool sums s = g0+g1+g2 per chunk (2 ucode TTs, standard library),
     DVE masks junk slots (TT vs a [128,16] residue tile broadcast-viewed
     over blocks) and compacts via a 4-level TT add tree.  cond =
     (cc >= 3*BAND)*CM; pair rows fold sigma = (cc - BAND*(cc>=BAND)) *
     VSELM (VSELM = is_pair + is_eye; sigma_eye decodes to exactly 1.0).
  6. Masks: Pool local_scatter for supertiles 0..SPLIT-1 (eye/pair rows
     scatter sv at up to 3 static cols, -1 padded), DVE for the rest via
     ONE tensor_tensor per supertile: bf16 PATTERN block (cast-loaded from
     fp8 DRAM) times sv broadcast-viewed along the column dim.
  7. Out DMAs alternate SP/Act HWDGE rings into a partition-major bf16
     DRAM shard; the host undoes the block interleave and casts to f32.
"""

import numpy as np
import ml_dtypes
from contextlib import ExitStack

import concourse.bacc as bacc
import concourse.tile as tile
from concourse import library_config, mybir
from concourse.bass_utils import run_bass_kernel_spmd
from concourse.tile_rust import add_dep_helper

# ---------------------------------------------------------------- constants
N = 128
D = 512
EPS = 32.0
SHARP = 10.0
NCORES = 8
BAND = 4.0                            # adjacency band scale in the g table
ZSLOT = 32 * 128                      # flat idx of the planted zero entry (32,0)

P_PAIRS = N * (N - 1) // 2            # 8128
T_TRIS = N * (N - 1) * (N - 2) // 6   # 341376
R_TOT = N + P_PAIRS + T_TRIS          # 349632
RC = R_TOT // NCORES                  # 43704 rows per core
NB = (RC + 127) // 128                # 342 blocks per core (last has 56 rows)
NBV = 65                              # blocks holding eye/pair rows (core 0)
SUPER = 16                            # blocks per supertile / output DMA
NSUP = (NB + SUPER - 1) // SUPER      # 22 (last has 6 blocks incl. tail)
SPLIT = 11                            # supertiles 0..SPLIT-1 Pool-scattered,
                                      # SPLIT..21 DVE pattern-multiplied
NBS = SPLIT * SUPER                   # scatter-path blocks
NBP = NB - NBS                        # pattern-path blocks

_DT = mybir.dt

# decode chunks: (block_lo, block_hi, has_pair_cols).  Chunk order feeds the
# Pool scatter stream first (supertiles 5.. are pure-triple on every core),
# then the eye/pair chunk, then the DVE pattern ranges.
# decode chunks (block ranges): A1/A2 feed the Pool scatter stream first
# (pure-triple on every core), B adds the eye/pair sigma chain, C/D cover
# the DVE pattern supertiles.
CH_A1 = (80, 128)         # supertiles 5..7
CH_A2 = (128, NBS)        # supertiles 8..SPLIT-1
CH_B = (0, 80)            # supertiles 0..4
CH_C = (NBS, 272)         # pattern supertiles SPLIT..16
CH_D = (272, NB)          # pattern supertiles 17..21
SCAT_ORDER = tuple(range(5, SPLIT)) + tuple(range(0, 5))


# ---------------------------------------------------------------- host tables
def _host_tables():
    """Static per-core tensors (independent of W)."""
    iu, ju = np.triu_indices(N, k=1)                      # pair lex order
    ti, tj, tk = [], [], []
    for i in range(N - 2):
        for j in range(i + 1, N - 1):
            ks = np.arange(j + 1, N)
            ti.append(np.full(len(ks), i))
            tj.append(np.full(len(ks), j))
            tk.append(ks)
    ti = np.concatenate(ti).astype(np.int64)
    tj = np.concatenate(tj).astype(np.int64)
    tk = np.concatenate(tk).astype(np.int64)
    assert ti.shape[0] == T_TRIS

    # global row r -> col indices (-1 = unused), selector classes, gather idx
    c1 = np.full(R_TOT, -1, np.int64)  # first one-hot col (i or eye col)
    c2 = np.full(R_TOT, -1, np.int64)  # second (j)
    c3 = np.full(R_TOT, -1, np.int64)  # third (k)
    cm = np.zeros(R_TOT, np.float32)   # 1 iff triple row
    vm = np.zeros(R_TOT, np.float32)   # 1 iff pair or eye row (sigma scale)
    ix1 = np.zeros(R_TOT, np.int16)    # flat idx into g for (i,j) (+pairs)
    ix2 = np.full(R_TOT, ZSLOT, np.int16)   # (j,k); zero slot elsewhere
    ix3 = np.full(R_TOT, ZSLOT, np.int16)   # (i,k); zero slot elsewhere

    r = np.arange(N)
    c1[:N] = r
    vm[:N] = 1.0                       # eye rows: sigma decodes to exactly 1.0

    s = N
    c1[s:s + P_PAIRS] = iu
    c2[s:s + P_PAIRS] = ju
    vm[s:s + P_PAIRS] = 1.0
    ix1[s:s + P_PAIRS] = (iu * N + ju).astype(np.int16)

    s = N + P_PAIRS
    c1[s:] = ti
    c2[s:] = tj
    c3[s:] = tk
    cm[s:] = 1.0
    ix1[s:] = (ti * N + tj).astype(np.int16)
    ix2[s:] = (tj * N + tk).astype(np.int16)
    ix3[s:] = (ti * N + tk).astype(np.int16)

    def shard(a, core, nb, fill=0):
        """rows [core*RC, core*RC+128*nb) -> [128, nb] (p, b), padded."""
        lo = core * RC
        take = min(RC, 128 * nb, a.shape[0] - lo)
        full = np.full(128 * nb, fill, a.dtype)
        full[:take] = a[lo:lo + take]
        return full.reshape(nb, 128).T.copy()

    def pattern(core):
        """[128, NBP*128] bf16: one-hot pattern for blocks NBS..NB."""
        lo = core * RC + NBS * 128
        nr = NBP * 128
        pat = np.zeros((nr, 128), np.float32)
        take = min(nr, R_TOT - lo)
        rr = np.arange(take)
        for cols in (c1, c2, c3):
            cc = cols[lo:lo + take]
            ok = cc >= 0
            pat[rr[ok], cc[ok]] = 1.0
        pat3 = pat.reshape(NBP, 128, 128).transpose(1, 0, 2)  # [p, b, c]
        segs = []
        for s_i in range(SPLIT, NSUP):
            b0 = s_i * SUPER - NBS
            b1 = min(NBP, b0 + SUPER)
            # per-supertile column-major [p, c, b]
            segs.append(np.ascontiguousarray(
                pat3[:, b0:b1, :].transpose(0, 2, 1)).reshape(128, -1))
        return np.concatenate(segs, axis=1).astype(ml_dtypes.bfloat16)

    def scatter_idx(core):
        """[128, 3*NBS] i16: scatter cols (local to 8-block groups) for
        blocks 0..NBS, -1 where the row has no such one-hot col."""
        lo = core * RC
        sidx = np.full((128, 3 * NBS), -1, np.int16)
        for t, cols in enumerate((c1, c2, c3)):
            cs = np.full(NB * 128, -1, np.int64)
            take = min(RC, c1.shape[0] - lo)
            cs[:take] = cols[lo:lo + take]
            cs = cs.reshape(NB, 128).T  # [p, b]
            for b in range(NBS):
                v = cs[:, b]
                loc = 128 * (b % 8) + v
                sidx[:, 3 * b + t] = np.where(v >= 0, loc, -1)
        return sidx

    per_core = []
    for c in range(NCORES):
        per_core.append({
            "PAT": pattern(c),
            "CM": shard(cm, c, NB).astype(ml_dtypes.bfloat16),
            "VSELM": shard(vm, c, NBV).astype(ml_dtypes.bfloat16),
            "IXALL": np.stack(
                [shard(ix1, c, NB), shard(ix2, c, NB, ZSLOT),
                 shard(ix3, c, NB, ZSLOT)],
                axis=2,
            ).reshape(128, 3 * NB),
            "SIDX": scatter_idx(c),
        })

    # residue-class mask, broadcast-viewed over blocks on device:
    # m16[p, r] = (p % 16 == r)
    m16 = ((np.arange(128) % 16)[:, None]
           == np.arange(16)[None, :]).astype(ml_dtypes.bfloat16)
    return per_core, None, m16


# ---------------------------------------------------------------- bass program
def _build_program():
    nc = bacc.Bacc(
        "TRN2", target_bir_lowering=False, debug=False,
        detect_race_conditions=False,
    )

    f32, bf16, u16 = _DT.float32, _DT.bfloat16, _DT.int16
    fp8 = _DT.float8e4
    WT_p = nc.declare_dram_parameter("WT", [N, D], f32, isOutput=False)
    WTM2_p = nc.declare_dram_parameter("WTM2", [N, D], f32, isOutput=False)
    SQP_p = nc.declare_dram_parameter("SQP", [128, 1], f32, isOutput=False)
    SQB_p = nc.declare_dram_parameter("SQB", [128, 128], f32, isOutput=False)
    PAT_p = nc.declare_dram_parameter("PAT", [128, NBP * 128], bf16, isOutput=False)
    BVD_p = nc.declare_dram_parameter("bvd", [N * N], f32, isOutput=True)
    CM_p = nc.declare_dram_parameter("CM", [128, NB], bf16, isOutput=False)
    VSELM_p = nc.declare_dram_parameter("VSELM", [128, NBV], bf16, isOutput=False)
    IXALL_p = nc.declare_dram_parameter("IXALL", [128, 3 * NB], u16, isOutput=False)
    M16_p = nc.declare_dram_parameter("M16", [128, 16], bf16, isOutput=False)
    SIDX_p = nc.declare_dram_parameter("SIDX", [128, 3 * NBS], u16, isOutput=False)
    # partition-major output: out[p, 128*b + c] = M[128*b + p, c]
    OUT_p = nc.declare_dram_parameter("out", [128, NB * 128], bf16, isOutput=True)

    mul, add_, ge = (
        mybir.AluOpType.mult, mybir.AluOpType.add, mybir.AluOpType.is_ge,
    )

    with tile.TileContext(nc) as tc, ExitStack() as ctx:
        const = ctx.enter_context(tc.tile_pool(name="const", bufs=1))
        work = ctx.enter_context(tc.tile_pool(name="work", bufs=1))
        psum = ctx.enter_context(tc.tile_pool(name="psum", bufs=1, space="PSUM"))
        psum2 = ctx.enter_context(tc.tile_pool(name="psum2", bufs=1, space="PSUM"))
        gpool = ctx.enter_context(tc.tile_pool(name="gath", bufs=1))
        pat = ctx.enter_context(tc.tile_pool(name="pat", bufs=4))
        sup = ctx.enter_context(tc.tile_pool(name="sup", bufs=4))

        nc.gpsimd.load_library(library_config.ap_gather)

        # prefetch the first pattern supertiles on the otherwise-idle Pool
        # ring (SWDGE) before its table-broadcast slice needs it
        pc_pref = {}
        for s_i in range(SPLIT, SPLIT + 4):
            b_lo, b_hi = s_i * SUPER, min(NB, (s_i + 1) * SUPER)
            psl = slice((b_lo - NBS) * 128, (b_hi - NBS) * 128)
            pc = pat.tile([128, SUPER * 128], bf16, name=f"pcp{s_i}", tag="pat")
            nc.gpsimd.dma_start(pc[:, :(b_hi - b_lo) * 128], PAT_p.ap()[:, psl])
            pc_pref[s_i] = pc

        load_instrs = {}

        def load(pool, param, shape, dt, eng=None):
            t = pool.tile(shape, dt, tag=param.name)
            load_instrs[param.name] = (eng or nc.sync).dma_start(t[:], param.ap())
            return t

        # input loads spread across SP and Act rings; dist operands first
        wt_sb = load(const, WT_p, [N, D], f32, nc.sync)
        wtm2_sb = load(const, WTM2_p, [N, D], f32, nc.scalar)
        sqp = load(const, SQP_p, [128, 1], f32, nc.sync)
        sqb = load(const, SQB_p, [128, 128], f32, nc.sync)
        ixall = load(const, IXALL_p, [128, 3 * NB], u16, nc.sync)
        sidx = load(const, SIDX_p, [128, 3 * NBS], u16, nc.scalar)
        cmt = load(const, CM_p, [128, NB], bf16, nc.sync)
        vselm = load(const, VSELM_p, [128, NBV], bf16, nc.scalar)
        m16t = load(const, M16_p, [128, 16], bf16, nc.sync)

        # preload the sigmoid act table off the critical path
        dum = work.tile([128, 1], f32)
        nc.vector.memset(dum[:], 0.25)
        nc.scalar.activation(dum[:], dum[:], mybir.ActivationFunctionType.Sigmoid)

        # ---- 1. d2 ---------------------------------------------------------
        gall = gpool.tile([128, 16 * 3 * NB], f32, tag="gall")
        d2 = psum.tile([128, 128], f32, tag="d2")
        for c4 in range(4):
            sl4 = slice(c4 * 128, (c4 + 1) * 128)
            nc.tensor.matmul(
                d2[:], wtm2_sb[:, sl4], wt_sb[:, sl4],
                start=(c4 == 0), stop=(c4 == 3)
            )

        # ---- 2. band table g = BAND*(d2<=eps^2) + sigma(d2), zero slot -----
        # d2 here is -2 W W^T; the +|w_i|^2 + |w_j|^2 terms come from the
        # host-marshalled sq vectors (per-partition scalar + broadcast rows)
        dfull = gall[:, D:D + 128]
        nc.vector.tensor_scalar(dfull, d2[:], sqp[:], None, add_)
        nc.vector.tensor_tensor(dfull, dfull, sqb[:], add_)
        bind = gall[:, D + 256:D + 384]
        nc.vector.tensor_scalar(bind, dfull, EPS * EPS, None, mybir.AluOpType.is_le)
        sigb = work.tile([128, 1], f32)
        nc.vector.memset(sigb[:], SHARP * EPS / 2.0)
        sgm = gall[:, D + 128:D + 256]
        nc.scalar.activation(
            sgm, dfull, mybir.ActivationFunctionType.Sigmoid,
            bias=sigb[:], scale=-SHARP / (2.0 * EPS),
        )
        bv = work.tile([128, 128], f32)
        nc.vector.scalar_tensor_tensor(bv[:], bind, BAND, sgm, mul, add_)
        # plant the zero slot at flat idx ZSLOT = (32, 0): lower triangle,
        # never referenced as a real pair (and a legal op start partition)
        nc.vector.memset(bv[32:33, 0:1], 0.0)

        # ---- 3. replicate bv to every partition via broadcast DMAs ---------
        bvd_w = nc.sync.dma_start(BVD_p.ap(), bv[:])
        bvtab = work.tile([128, N * N], f32, tag="bvtab")
        bcast_engs = (nc.sync, nc.scalar, nc.gpsimd)
        bcast_cuts = (0, 5764, 10412, N * N)
        bcast_loads = []
        for q in range(3):
            sl = slice(bcast_cuts[q], bcast_cuts[q + 1])
            bl = bcast_engs[q].dma_start(
                bvtab[:, sl], BVD_p.ap()[sl].partition_broadcast(128)
            )
            add_dep_helper(bl.ins, bvd_w.ins, reason="table RAW via DRAM")
            bcast_loads.append(bl)

        # ---- 4. gather: g at (i,j), (j,k), (i,k) per output row ------------
        gi = nc.gpsimd.ap_gather(
            gall[:], bvtab[:], ixall[:],
            channels=128, num_elems=N * N, d=1, num_idxs=16 * 3 * NB,
        )
        for bl in bcast_loads:
            add_dep_helper(gi.ins, bl.ins, reason="gather after table")
        add_dep_helper(gi.ins, load_instrs["IXALL"].ins, reason="gather after idx")

        gv = gall[:].rearrange("p (b t r) -> p b t r", t=3, r=16)
        st = work.tile([128, 16 * NB], bf16, tag="st")
        stv = st[:].rearrange("p (b r) -> p b r", r=16)
        sv = work.tile([128, NB], bf16)
        cc = work.tile([128, NB], bf16)
        tmp = work.tile([128, NB], bf16)
        sdata = work.tile([128, 3 * NBS], bf16)
        sig = work.tile([128, NBV], bf16)
        siga = work.tile([128, NBV], bf16)

        # ---- 5. decode: Pool sums A/B, DVE sums C/D, DVE masks + compacts --
        lls = nc.gpsimd.load_library(library_config.standard)
        add_dep_helper(lls.ins, gi.ins, reason="lib switch after gather")

        def sum_chunk(lo, hi, eng):
            svw = stv[:, lo:hi, :]
            s1 = eng.tensor_tensor(
                svw, gv[:, lo:hi, 0, :], gv[:, lo:hi, 1, :], add_)
            add_dep_helper(s1.ins, gi.ins, reason="sum after gather")
            add_dep_helper(s1.ins, lls.ins, reason="sum after lib")
            s2 = eng.tensor_tensor(svw, svw, gv[:, lo:hi, 2, :], add_)
            add_dep_helper(s2.ins, gi.ins, reason="sum after gather")
            return s2

        def dve_compact_chunk(lo, hi, has_pair, sum_op):
            n = hi - lo
            svw = stv[:, lo:hi, :]
            m16v = m16t[:].unsqueeze(1).to_broadcast([128, n, 16])
            with nc.allow_low_precision(reason="one-hot residue tree"):
                mk = nc.vector.tensor_tensor(svw, svw, m16v, mul)
                add_dep_helper(mk.ins, sum_op.ins, reason="mask after sum")
                # 4-level TT add tree: 16 -> 8 -> 4 -> 2 -> 1 slots
                for half in (8, 4, 2):
                    nc.vector.tensor_tensor(
                        svw[:, :, 0:half], svw[:, :, 0:half],
                        svw[:, :, half:2 * half], add_,
                    )
                nc.vector.tensor_tensor(
                    cc[:, lo:hi].unsqueeze(2),
                    svw[:, :, 0:1], svw[:, :, 1:2], add_,
                )
            # cond = (cc >= 3*BAND), sv = cond * CM
            nc.vector.tensor_scalar(
                tmp[:, lo:hi], cc[:, lo:hi], 3.0 * BAND, None, ge)
            nc.vector.tensor_tensor(
                sv[:, lo:hi], tmp[:, lo:hi], cmt[:, lo:hi], mul)
            if has_pair:
                # pair rows: cc = g0 = BAND*A0 + sigma (2nd/3rd idx hit the
                # planted zero slot).  sigma = cc - BAND*(cc >= BAND); eye
                # rows decode to exactly 1.0, pads to 0.
                nc.vector.tensor_scalar(siga[:], cc[:, :NBV], BAND, None, ge)
                nc.vector.scalar_tensor_tensor(
                    sig[:], siga[:], -BAND, cc[:, :NBV], mul, add_)
                nc.vector.tensor_tensor(sig[:], sig[:], vselm[:], mul)
                nc.vector.tensor_tensor(
                    sv[:, :NBV], sv[:, :NBV], sig[:], add_)

        def sdata_chunk(lo, hi):
            # sdata[p, 3b+t] = sv[p, b]: one TS with a broadcast view
            svv = sv[:, lo:hi].unsqueeze(2).to_broadcast([128, hi - lo, 3])
            return nc.vector.tensor_scalar(
                sdata[:].rearrange("p (b t) -> p b t", t=3)[:, lo:hi, :],
                svv, 1.0, None, mul,
            )

        # ---- 6a. Pool-scattered supertiles ---------------------------------
        scat = [
            gpool.tile([128, SUPER * 128], bf16, name=f"scat{i}", tag=f"scat{i}")
            for i in range(4)
        ]
        sdata_ops = {}
        scat_prev = {}

        def scatter_super(s_i, slot, ll2):
            b_lo = s_i * SUPER
            csl = slice(b_lo * 128, (b_lo + SUPER) * 128)
            stt_ = scat[slot]
            scs = []
            for h in range(2):
                a0 = b_lo + 8 * h
                sc = nc.gpsimd.local_scatter(
                    stt_[:, h * 1024:(h + 1) * 1024],
                    sdata[:, 3 * a0:3 * (a0 + 8)],
                    sidx[:, 3 * a0:3 * (a0 + 8)],
                    channels=128, num_elems=1024, num_idxs=24,
                )
                add_dep_helper(sc.ins, ll2.ins, reason="scatter after lib")
                add_dep_helper(
                    sc.ins, sdata_ops[s_i].ins, reason="scatter after sdata")
                add_dep_helper(
                    sc.ins, load_instrs["SIDX"].ins, reason="scatter after idx")
                if slot in scat_prev:
                    add_dep_helper(
                        sc.ins, scat_prev[slot].ins, reason="scatter buf WAR")
                scs.append(sc)
            eng = nc.sync if s_i % 2 == 0 else nc.scalar
            dma = eng.dma_start(OUT_p.ap()[:, csl], stt_[:])
            for sc in scs:
                add_dep_helper(dma.ins, sc.ins, reason="dma after scatter")
            scat_prev[slot] = dma

        # ---- 6b. DVE pattern supertiles ------------------------------------
        def mask_super(s_i):
            b_lo = s_i * SUPER
            b_hi = min(NB, b_lo + SUPER)
            nblk = b_hi - b_lo
            csl = slice(b_lo * 128, b_hi * 128)
            psl = slice((b_lo - NBS) * 128, (b_hi - NBS) * 128)
            pc = pc_pref[s_i]
            stt_ = sup.tile([128, SUPER * 128], bf16, tag="super")
            # PAT and the output region are COLUMN-major per supertile
            # ([p, c, b]); sv then broadcasts over the MIDDLE dim with its
            # own packed last dim, so one all-bf16 TT runs in the 0.5x mode.
            # The host unshards the transposed region.
            svv = (sv[:, b_lo:b_hi].unsqueeze(1)
                   .to_broadcast([128, 128, nblk]))
            nc.vector.tensor_tensor(
                stt_[:, :nblk * 128].rearrange("p (c b) -> p c b", b=nblk),
                pc[:, :nblk * 128].rearrange("p (c b) -> p c b", b=nblk),
                svv, mul,
            )
            if s_i >= NSUP - 2:
                eng2 = nc.gpsimd   # Pool ring is idle by the time these run
            else:
                eng2 = nc.scalar if s_i % 2 == 0 else nc.sync
            eng2.dma_start(OUT_p.ap()[:, csl], stt_[:, :nblk * 128])

        # pattern supertiles 15..21 overwrite the (dead after the gather)
        # broadcast table: bitcast bf16 views of bvtab, loaded while the
        # SP/Act rings are otherwise idle right after the gather
        for k, s_i in enumerate(range(SPLIT + 4, NSUP)):
            b_lo, b_hi = s_i * SUPER, min(NB, (s_i + 1) * SUPER)
            psl = slice((b_lo - NBS) * 128, (b_hi - NBS) * 128)
            pcv = bvtab[:, k * 1024:(k + 1) * 1024].bitcast(bf16)
            eng = nc.sync if s_i % 2 == 0 else nc.scalar
            pcd = eng.dma_start(pcv[:, :(b_hi - b_lo) * 128], PAT_p.ap()[:, psl])
            add_dep_helper(pcd.ins, gi.ins, reason="pat overwrites table WAR")
            pc_pref[s_i] = pcv

        # ---- schedule -------------------------------------------------------
        # DVE sums chunk A1 itself (its first post-gather work), so Pool's
        # queue is just [sums A2/B/C/D, lib, scatters] and the scatter
        # stream starts as soon as DVE posts sdata A1.
        sum_a1 = sum_chunk(*CH_A1, nc.vector)
        sum_a2 = sum_chunk(*CH_A2, nc.gpsimd)
        sum_b = sum_chunk(*CH_B, nc.gpsimd)
        sum_c = sum_chunk(*CH_C, nc.gpsimd)
        sum_d = sum_chunk(*CH_D, nc.gpsimd)
        ll2 = nc.gpsimd.load_library(library_config.local_scatter)
        add_dep_helper(ll2.ins, sum_d.ins, reason="lib switch after sums")

        scat_i = 0

        def emit_scatters(n):
            nonlocal scat_i
            for _ in range(n):
                if scat_i >= len(SCAT_ORDER):
                    return
                s_i = SCAT_ORDER[scat_i]
                scatter_super(s_i, scat_i % 4, ll2)
                scat_i += 1

        for (lo, hi), has_pair, sum_op in (
            (CH_A1, False, sum_a1), (CH_A2, False, sum_a2),
            (CH_B, True, sum_b),
        ):
            dve_compact_chunk(lo, hi, has_pair, sum_op)
            op = sdata_chunk(lo, hi)
            for s_i in range(lo // SUPER, hi // SUPER):
                sdata_ops[s_i] = op
            emit_scatters((hi - lo) // SUPER)
        dve_compact_chunk(*CH_C, False, sum_c)
        for s_i in range(SPLIT, (CH_C[1] + SUPER - 1) // SUPER):
            mask_super(s_i)
        dve_compact_chunk(*CH_D, False, sum_d)
        for s_i in range(17, NSUP):
            mask_super(s_i)
        emit_scatters(len(SCAT_ORDER))

    nc.compile()
    return nc


_PROGRAM = None
_TABLES = None


def _get_program():
    global _PROGRAM, _TABLES
    if _PROGRAM is None:
        _TABLES = _host_tables()
        _PROGRAM = _build_program()
    return _PROGRAM, _TABLES


def _feeds(core, W, per_core, ident, m16):
    t = per_core[core]
    wt = np.ascontiguousarray(
        W.T.reshape(4, 128, 128).transpose(1, 0, 2).reshape(128, 512)
    )
    sq = (W * W).sum(axis=1).astype(np.float32)
    return {
        "WT": wt, "WTM2": np.ascontiguousarray(-2.0 * wt),
        "SQP": np.ascontiguousarray(sq.reshape(128, 1)),
        "SQB": np.ascontiguousarray(np.broadcast_to(sq, (128, 128))),
        "M16": m16,
        "PAT": t["PAT"], "CM": t["CM"], "VSELM": t["VSELM"],
        "IXALL": t["IXALL"], "SIDX": t["SIDX"],
    }


def _unshard(out_pm: np.ndarray) -> np.ndarray:
    """[128, NB*128] partition-major bf16 -> [RC, 128] f32.

    Scatter supertiles (blocks < NBS) are [p, b, c]; pattern supertiles are
    column-major [p, c, b] per supertile.
    """
    out_pm = out_pm.astype(np.float32)
    rows = np.empty((NB * 128, 128), np.float32)
    rows[:NBS * 128] = (
        out_pm[:, :NBS * 128].reshape(128, NBS, 128)
        .transpose(1, 0, 2).reshape(NBS * 128, 128)
    )
    for s_i in range(SPLIT, NSUP):
        b_lo = s_i * SUPER
        b_hi = min(NB, b_lo + SUPER)
        nblk = b_hi - b_lo
        seg = out_pm[:, b_lo * 128:b_hi * 128].reshape(128, 128, nblk)
        rows[b_lo * 128:b_hi * 128] = (
            seg.transpose(2, 0, 1).reshape(nblk * 128, 128)
        )
    return rows[:RC]


def kernel(W: np.ndarray) -> np.ndarray:
    nc, (per_core, ident, m16) = _get_program()
    W = np.ascontiguousarray(np.asarray(W, dtype=np.float32))
    in_maps = [_feeds(c, W, per_core, ident, m16) for c in range(NCORES)]
    res = run_bass_kernel_spmd(nc, in_maps, list(range(NCORES)))
    shards = [_unshard(np.asarray(res.results[c]["out"])) for c in range(NCORES)]
    return np.concatenate(shards, axis=0)


# revision 38
# speedup vs baseline: 1.7601x; 1.0278x over previous
"""Trainium2 Bass kernel for DifferentiableVietorisRips.

Output M = concat([eye(N); pair-masks; triple-masks]) with
  N = 128, D = 512, EPSILON = 32.0, SHARPNESS = 10.0, VR_DIM = 2
  pair rows   : P = C(128,2) = 8128,  row(i,j) has sigmoid(10*(32-d_ij)) at cols i,j
  triple rows : T = C(128,3) = 341376, row(i,j,k) has [d_ij<=32 & d_jk<=32 & d_ik<=32]
                at cols i,j,k
  M shape [349632, 128] float32.

Sharding: contiguous row chunks, 43704 rows/core across 8 cores. One uniform
SPMD Bass program; every per-core difference lives in input tensors.

v5 pipeline, CoreSim 59.0us (baseline rewrite was 103.9us):
  1. d2 = -2 W W^T via PE (f32); the +|wi|^2+|wj|^2 terms come from host-
     marshalled sq vectors (per-partition TS scalar + broadcast-row TT) --
     no on-device row-reduce, no aug transposes.  No sqrt anywhere: the
     adjacency bit compares in the squared domain and sigma =
     sigmoid(SHARP/(2 eps) * (eps^2 - d2)) matches sigmoid(SHARP*(eps-d))
     to ~7e-4 (first-order at threshold, saturated elsewhere), so ONE
     activation table, preloaded by a dummy op at t~0.
  2. Band table g = BAND*(adj) + sigma with BAND=4: the triple-AND becomes
     a SUM test (g0+g1+g2 >= 12, sigma < 1 keeps two-bit rows < 11).
     g[32,0] (lower triangle, never referenced) is memset to 0; pair/eye
     rows point their 2nd/3rd gather index there so pair sigma = cc -
     BAND*(cc>=BAND) decodes without bf16 cancellation (eye rows give
     exactly 1.0, pads 0 -- one VSELM table handles all three).
  3. g -> DRAM row, then three partition-broadcast DMAs (SP/Act/Pool,
     stride-0 DRAM source, sizes tuned so all sems land when Pool's own
     slice ends) replicate it to bvtab [128,16384] f32 in ~9us.
  4. One merged ap_gather (16416 slots = exactly the per-group lookup
     count, ~13.7us Pool) fetches g(i,j), g(j,k), g(i,k) per row in
     (block, t, residue) slot nesting.
  5. Decode, engine-split by measured cost-model modes (DVE TT/TS get the
     0.5x mode only when no operand is f32-strided-broadcast or fp8;
     TensorReduce and scalar_tensor_tensor always run 1.0x and are
     avoided; Pool ucode TT = 0.83ns/elem flat): DVE sums chunk A1 then
     masks/compacts each chunk as its sum lands; Pool (standard library)
     sums chunks A2/B/C/D.  Junk slots die by a TT against a [128,16]
     residue tile broadcast-viewed over blocks (middle-dim broadcast keeps
     the fast mode); a 4-level TT add tree (16->1) replaces TensorReduce
     at half the cost.  cond = (cc >= 3*BAND) * CM + sigma * VSELM.
  6. Masks, three producers in parallel:
     - supertiles 0..SPLIT-1: Pool local_scatter (eye/pair rows scatter sv
       at up to 3 static cols, -1 padded), 4 rotating buffers;
     - supertiles SPLIT..21: ONE all-bf16 TT per supertile, PAT block
       times sv broadcast over the middle dim -- PAT and the DRAM region
       are column-major [p, c, b] per supertile (host unshards);
       supertiles >= FP8S write fp8 (exact: pure 0/1) to halve their DMA.
     - PAT staging: 4 supertiles prefetched on the idle Pool ring before
       its broadcast slice, 1 on SP during the gather, the rest overwrite
       the dead bvtab (bitcast bf16 views) right after the gather.
  7. Out DMAs alternate SP/Act rings (the last two ride the by-then idle
     Pool ring) into partition-major bf16/fp8 DRAM shards; the host
     undoes the interleaves and casts to f32.
"""

import numpy as np
import ml_dtypes
from contextlib import ExitStack

import concourse.bacc as bacc
import concourse.tile as tile
from concourse import library_config, mybir
from concourse.bass_utils import run_bass_kernel_spmd
from concourse.tile_rust import add_dep_helper

# ---------------------------------------------------------------- constants
N = 128
D = 512
EPS = 32.0
SHARP = 10.0
NCORES = 8
BAND = 4.0                            # adjacency band scale in the g table
ZSLOT = 32 * 128                      # flat idx of the planted zero entry (32,0)

P_PAIRS = N * (N - 1) // 2            # 8128
T_TRIS = N * (N - 1) * (N - 2) // 6   # 341376
R_TOT = N + P_PAIRS + T_TRIS          # 349632
RC = R_TOT // NCORES                  # 43704 rows per core
NB = (RC + 127) // 128                # 342 blocks per core (last has 56 rows)
NBV = 65                              # blocks holding eye/pair rows (core 0)
SUPER = 16                            # blocks per supertile / output DMA
NSUP = (NB + SUPER - 1) // SUPER      # 22 (last has 6 blocks incl. tail)
SPLIT = 10                            # supertiles 0..SPLIT-1 Pool-scattered,
                                      # SPLIT..21 DVE pattern-multiplied
NBS = SPLIT * SUPER                   # scatter-path blocks
NBP = NB - NBS                        # pattern-path blocks
FP8S = 20                             # first fp8-output supertile

_DT = mybir.dt

# decode chunks: (block_lo, block_hi, has_pair_cols).  Chunk order feeds the
# Pool scatter stream first (supertiles 5.. are pure-triple on every core),
# then the eye/pair chunk, then the DVE pattern ranges.
# decode chunks (block ranges): A1/A2 feed the Pool scatter stream first
# (pure-triple on every core), B adds the eye/pair sigma chain, C/D cover
# the DVE pattern supertiles.
CH_A1 = (80, 128)         # supertiles 5..7
CH_A2 = (128, NBS)        # supertiles 8..SPLIT-1
CH_B = (0, 80)            # supertiles 0..4
CH_C = (NBS, 272)         # pattern supertiles SPLIT..16
CH_D = (272, NB)          # pattern supertiles 17..21
SCAT_ORDER = tuple(range(5, SPLIT)) + tuple(range(0, 5))


# ---------------------------------------------------------------- host tables
def _host_tables():
    """Static per-core tensors (independent of W)."""
    iu, ju = np.triu_indices(N, k=1)                      # pair lex order
    ti, tj, tk = [], [], []
    for i in range(N - 2):
        for j in range(i + 1, N - 1):
            ks = np.arange(j + 1, N)
            ti.append(np.full(len(ks), i))
            tj.append(np.full(len(ks), j))
            tk.append(ks)
    ti = np.concatenate(ti).astype(np.int64)
    tj = np.concatenate(tj).astype(np.int64)
    tk = np.concatenate(tk).astype(np.int64)
    assert ti.shape[0] == T_TRIS

    # global row r -> col indices (-1 = unused), selector classes, gather idx
    c1 = np.full(R_TOT, -1, np.int64)  # first one-hot col (i or eye col)
    c2 = np.full(R_TOT, -1, np.int64)  # second (j)
    c3 = np.full(R_TOT, -1, np.int64)  # third (k)
    cm = np.zeros(R_TOT, np.float32)   # 1 iff triple row
    vm = np.zeros(R_TOT, np.float32)   # 1 iff pair or eye row (sigma scale)
    ix1 = np.zeros(R_TOT, np.int16)    # flat idx into g for (i,j) (+pairs)
    ix2 = np.full(R_TOT, ZSLOT, np.int16)   # (j,k); zero slot elsewhere
    ix3 = np.full(R_TOT, ZSLOT, np.int16)   # (i,k); zero slot elsewhere

    r = np.arange(N)
    c1[:N] = r
    vm[:N] = 1.0                       # eye rows: sigma decodes to exactly 1.0

    s = N
    c1[s:s + P_PAIRS] = iu
    c2[s:s + P_PAIRS] = ju
    vm[s:s + P_PAIRS] = 1.0
    ix1[s:s + P_PAIRS] = (iu * N + ju).astype(np.int16)

    s = N + P_PAIRS
    c1[s:] = ti
    c2[s:] = tj
    c3[s:] = tk
    cm[s:] = 1.0
    ix1[s:] = (ti * N + tj).astype(np.int16)
    ix2[s:] = (tj * N + tk).astype(np.int16)
    ix3[s:] = (ti * N + tk).astype(np.int16)

    def shard(a, core, nb, fill=0):
        """rows [core*RC, core*RC+128*nb) -> [128, nb] (p, b), padded."""
        lo = core * RC
        take = min(RC, 128 * nb, a.shape[0] - lo)
        full = np.full(128 * nb, fill, a.dtype)
        full[:take] = a[lo:lo + take]
        return full.reshape(nb, 128).T.copy()

    def pattern(core):
        """[128, NBP*128] bf16: one-hot pattern for blocks NBS..NB."""
        lo = core * RC + NBS * 128
        nr = NBP * 128
        pat = np.zeros((nr, 128), np.float32)
        take = min(nr, R_TOT - lo)
        rr = np.arange(take)
        for cols in (c1, c2, c3):
            cc = cols[lo:lo + take]
            ok = cc >= 0
            pat[rr[ok], cc[ok]] = 1.0
        pat3 = pat.reshape(NBP, 128, 128).transpose(1, 0, 2)  # [p, b, c]
        segs = []
        for s_i in range(SPLIT, NSUP):
            b0 = s_i * SUPER - NBS
            b1 = min(NBP, b0 + SUPER)
            # per-supertile column-major [p, c, b]
            segs.append(np.ascontiguousarray(
                pat3[:, b0:b1, :].transpose(0, 2, 1)).reshape(128, -1))
        return np.concatenate(segs, axis=1).astype(ml_dtypes.bfloat16)

    def scatter_idx(core):
        """[128, 3*NBS] i16: scatter cols (local to 8-block groups) for
        blocks 0..NBS, -1 where the row has no such one-hot col."""
        lo = core * RC
        sidx = np.full((128, 3 * NBS), -1, np.int16)
        for t, cols in enumerate((c1, c2, c3)):
            cs = np.full(NB * 128, -1, np.int64)
            take = min(RC, c1.shape[0] - lo)
            cs[:take] = cols[lo:lo + take]
            cs = cs.reshape(NB, 128).T  # [p, b]
            for b in range(NBS):
                v = cs[:, b]
                loc = 128 * (b % 8) + v
                sidx[:, 3 * b + t] = np.where(v >= 0, loc, -1)
        return sidx

    per_core = []
    for c in range(NCORES):
        per_core.append({
            "PAT": pattern(c),
            "CM": shard(cm, c, NB).astype(ml_dtypes.bfloat16),
            "VSELM": shard(vm, c, NBV).astype(ml_dtypes.bfloat16),
            "IXALL": np.stack(
                [shard(ix1, c, NB), shard(ix2, c, NB, ZSLOT),
                 shard(ix3, c, NB, ZSLOT)],
                axis=2,
            ).reshape(128, 3 * NB),
            "SIDX": scatter_idx(c),
        })

    # residue-class mask, broadcast-viewed over blocks on device:
    # m16[p, r] = (p % 16 == r)
    m16 = ((np.arange(128) % 16)[:, None]
           == np.arange(16)[None, :]).astype(ml_dtypes.bfloat16)
    return per_core, None, m16


# ---------------------------------------------------------------- bass program
def _build_program():
    nc = bacc.Bacc(
        "TRN2", target_bir_lowering=False, debug=False,
        detect_race_conditions=False,
    )

    f32, bf16, u16 = _DT.float32, _DT.bfloat16, _DT.int16
    fp8 = _DT.float8e4
    WT_p = nc.declare_dram_parameter("WT", [N, D], f32, isOutput=False)
    WTM2_p = nc.declare_dram_parameter("WTM2", [N, D], f32, isOutput=False)
    SQP_p = nc.declare_dram_parameter("SQP", [128, 1], f32, isOutput=False)
    SQB_p = nc.declare_dram_parameter("SQB", [128, 128], f32, isOutput=False)
    PAT_p = nc.declare_dram_parameter("PAT", [128, NBP * 128], bf16, isOutput=False)
    BVD_p = nc.declare_dram_parameter("bvd", [N * N], f32, isOutput=True)
    CM_p = nc.declare_dram_parameter("CM", [128, NB], bf16, isOutput=False)
    VSELM_p = nc.declare_dram_parameter("VSELM", [128, NBV], bf16, isOutput=False)
    IXALL_p = nc.declare_dram_parameter("IXALL", [128, 3 * NB], u16, isOutput=False)
    M16_p = nc.declare_dram_parameter("M16", [128, 16], bf16, isOutput=False)
    SIDX_p = nc.declare_dram_parameter("SIDX", [128, 3 * NBS], u16, isOutput=False)
    # partition-major output: out[p, 128*b + c] = M[128*b + p, c].
    # Supertiles >= FP8S hold only 0/1 cond values: exact in fp8, half the
    # DMA bytes right where the rings and the Pool tail are saturated.
    OUT_p = nc.declare_dram_parameter("out", [128, NB * 128], bf16, isOutput=True)
    OUT2_p = nc.declare_dram_parameter(
        "out2", [128, (NB - FP8S * SUPER) * 128], fp8, isOutput=True)

    mul, add_, ge = (
        mybir.AluOpType.mult, mybir.AluOpType.add, mybir.AluOpType.is_ge,
    )

    with tile.TileContext(nc) as tc, ExitStack() as ctx:
        const = ctx.enter_context(tc.tile_pool(name="const", bufs=1))
        work = ctx.enter_context(tc.tile_pool(name="work", bufs=1))
        psum = ctx.enter_context(tc.tile_pool(name="psum", bufs=1, space="PSUM"))
        psum2 = ctx.enter_context(tc.tile_pool(name="psum2", bufs=1, space="PSUM"))
        gpool = ctx.enter_context(tc.tile_pool(name="gath", bufs=1))
        pat = ctx.enter_context(tc.tile_pool(name="pat", bufs=5))
        sup = ctx.enter_context(tc.tile_pool(name="sup", bufs=5))

        nc.gpsimd.load_library(library_config.ap_gather)

        # prefetch the first pattern supertiles on the otherwise-idle Pool
        # ring (SWDGE) before its table-broadcast slice needs it
        pc_pref = {}
        for s_i in range(SPLIT, SPLIT + 4):
            b_lo, b_hi = s_i * SUPER, min(NB, (s_i + 1) * SUPER)
            psl = slice((b_lo - NBS) * 128, (b_hi - NBS) * 128)
            pc = pat.tile([128, SUPER * 128], bf16, name=f"pcp{s_i}", tag="pat")
            nc.gpsimd.dma_start(pc[:, :(b_hi - b_lo) * 128], PAT_p.ap()[:, psl])
            pc_pref[s_i] = pc

        load_instrs = {}

        def load(pool, param, shape, dt, eng=None):
            t = pool.tile(shape, dt, tag=param.name)
            load_instrs[param.name] = (eng or nc.sync).dma_start(t[:], param.ap())
            return t

        # input loads spread across SP and Act rings; dist operands first
        wt_sb = load(const, WT_p, [N, D], f32, nc.sync)
        wtm2_sb = load(const, WTM2_p, [N, D], f32, nc.scalar)
        sqp = load(const, SQP_p, [128, 1], f32, nc.sync)
        sqb = load(const, SQB_p, [128, 128], f32, nc.sync)
        ixall = load(const, IXALL_p, [128, 3 * NB], u16, nc.sync)
        sidx = load(const, SIDX_p, [128, 3 * NBS], u16, nc.scalar)
        cmt = load(const, CM_p, [128, NB], bf16, nc.sync)
        vselm = load(const, VSELM_p, [128, NBV], bf16, nc.scalar)
        m16t = load(const, M16_p, [128, 16], bf16, nc.sync)

        # preload the sigmoid act table off the critical path
        dum = work.tile([128, 1], f32)
        nc.vector.memset(dum[:], 0.25)
        nc.scalar.activation(dum[:], dum[:], mybir.ActivationFunctionType.Sigmoid)

        # ---- 1. d2 ---------------------------------------------------------
        gall = gpool.tile([128, 16 * 3 * NB], f32, tag="gall")
        d2 = psum.tile([128, 128], f32, tag="d2")
        for c4 in range(4):
            sl4 = slice(c4 * 128, (c4 + 1) * 128)
            nc.tensor.matmul(
                d2[:], wtm2_sb[:, sl4], wt_sb[:, sl4],
                start=(c4 == 0), stop=(c4 == 3)
            )

        # ---- 2. band table g = BAND*(d2<=eps^2) + sigma(d2), zero slot -----
        # d2 here is -2 W W^T; the +|w_i|^2 + |w_j|^2 terms come from the
        # host-marshalled sq vectors (per-partition scalar + broadcast rows)
        dfull = gall[:, D:D + 128]
        nc.vector.tensor_scalar(dfull, d2[:], sqp[:], None, add_)
        nc.vector.tensor_tensor(dfull, dfull, sqb[:], add_)
        bind = gall[:, D + 256:D + 384]
        nc.vector.tensor_scalar(bind, dfull, EPS * EPS, None, mybir.AluOpType.is_le)
        sigb = work.tile([128, 1], f32)
        nc.vector.memset(sigb[:], SHARP * EPS / 2.0)
        sgm = gall[:, D + 128:D + 256]
        nc.scalar.activation(
            sgm, dfull, mybir.ActivationFunctionType.Sigmoid,
            bias=sigb[:], scale=-SHARP / (2.0 * EPS),
        )
        bv = work.tile([128, 128], f32)
        nc.vector.scalar_tensor_tensor(bv[:], bind, BAND, sgm, mul, add_)
        # plant the zero slot at flat idx ZSLOT = (32, 0): lower triangle,
        # never referenced as a real pair (and a legal op start partition)
        nc.vector.memset(bv[32:33, 0:1], 0.0)

        # ---- 3. replicate bv to every partition via broadcast DMAs ---------
        bvd_w = nc.sync.dma_start(BVD_p.ap(), bv[:])
        bvtab = work.tile([128, N * N], f32, tag="bvtab")
        bcast_engs = (nc.sync, nc.scalar, nc.gpsimd)
        bcast_cuts = (0, 5764, 10412, N * N)
        bcast_loads = []
        for q in range(3):
            sl = slice(bcast_cuts[q], bcast_cuts[q + 1])
            bl = bcast_engs[q].dma_start(
                bvtab[:, sl], BVD_p.ap()[sl].partition_broadcast(128)
            )
            add_dep_helper(bl.ins, bvd_w.ins, reason="table RAW via DRAM")
            bcast_loads.append(bl)

        # ---- 4. gather: g at (i,j), (j,k), (i,k) per output row ------------
        gi = nc.gpsimd.ap_gather(
            gall[:], bvtab[:], ixall[:],
            channels=128, num_elems=N * N, d=1, num_idxs=16 * 3 * NB,
        )
        for bl in bcast_loads:
            add_dep_helper(gi.ins, bl.ins, reason="gather after table")
        add_dep_helper(gi.ins, load_instrs["IXALL"].ins, reason="gather after idx")

        gv = gall[:].rearrange("p (b t r) -> p b t r", t=3, r=16)
        st = work.tile([128, 16 * NB], bf16, tag="st")
        stv = st[:].rearrange("p (b r) -> p b r", r=16)
        sv = work.tile([128, NB], bf16)
        cc = work.tile([128, NB], bf16)
        tmp = work.tile([128, NB], bf16)
        sdata = work.tile([128, 3 * NBS], bf16)
        sig = work.tile([128, NBV], bf16)
        siga = work.tile([128, NBV], bf16)

        # ---- 5. decode: Pool sums A/B, DVE sums C/D, DVE masks + compacts --
        lls = nc.gpsimd.load_library(library_config.standard)
        add_dep_helper(lls.ins, gi.ins, reason="lib switch after gather")

        def sum_chunk(lo, hi, eng):
            svw = stv[:, lo:hi, :]
            s1 = eng.tensor_tensor(
                svw, gv[:, lo:hi, 0, :], gv[:, lo:hi, 1, :], add_)
            add_dep_helper(s1.ins, gi.ins, reason="sum after gather")
            add_dep_helper(s1.ins, lls.ins, reason="sum after lib")
            s2 = eng.tensor_tensor(svw, svw, gv[:, lo:hi, 2, :], add_)
            add_dep_helper(s2.ins, gi.ins, reason="sum after gather")
            return s2

        def dve_compact_chunk(lo, hi, has_pair, sum_op):
            n = hi - lo
            svw = stv[:, lo:hi, :]
            m16v = m16t[:].unsqueeze(1).to_broadcast([128, n, 16])
            with nc.allow_low_precision(reason="one-hot residue tree"):
                mk = nc.vector.tensor_tensor(svw, svw, m16v, mul)
                add_dep_helper(mk.ins, sum_op.ins, reason="mask after sum")
                # 4-level TT add tree: 16 -> 8 -> 4 -> 2 -> 1 slots
                for half in (8, 4, 2):
                    nc.vector.tensor_tensor(
                        svw[:, :, 0:half], svw[:, :, 0:half],
                        svw[:, :, half:2 * half], add_,
                    )
                nc.vector.tensor_tensor(
                    cc[:, lo:hi].unsqueeze(2),
                    svw[:, :, 0:1], svw[:, :, 1:2], add_,
                )
            # cond = (cc >= 3*BAND), sv = cond * CM
            nc.vector.tensor_scalar(
                tmp[:, lo:hi], cc[:, lo:hi], 3.0 * BAND, None, ge)
            nc.vector.tensor_tensor(
                sv[:, lo:hi], tmp[:, lo:hi], cmt[:, lo:hi], mul)
            if has_pair:
                # pair rows: cc = g0 = BAND*A0 + sigma (2nd/3rd idx hit the
                # planted zero slot).  sigma = cc - BAND*(cc >= BAND); eye
                # rows decode to exactly 1.0, pads to 0.
                nc.vector.tensor_scalar(siga[:], cc[:, :NBV], BAND, None, ge)
                nc.vector.scalar_tensor_tensor(
                    sig[:], siga[:], -BAND, cc[:, :NBV], mul, add_)
                nc.vector.tensor_tensor(sig[:], sig[:], vselm[:], mul)
                nc.vector.tensor_tensor(
                    sv[:, :NBV], sv[:, :NBV], sig[:], add_)

        def sdata_chunk(lo, hi):
            # sdata[p, 3b+t] = sv[p, b]: one TS with a broadcast view
            svv = sv[:, lo:hi].unsqueeze(2).to_broadcast([128, hi - lo, 3])
            return nc.vector.tensor_scalar(
                sdata[:].rearrange("p (b t) -> p b t", t=3)[:, lo:hi, :],
                svv, 1.0, None, mul,
            )

        # ---- 6a. Pool-scattered supertiles ---------------------------------
        scat = [
            gpool.tile([128, SUPER * 128], bf16, name=f"scat{i}", tag=f"scat{i}")
            for i in range(4)
        ]
        sdata_ops = {}
        scat_prev = {}

        def scatter_super(s_i, slot, ll2):
            b_lo = s_i * SUPER
            csl = slice(b_lo * 128, (b_lo + SUPER) * 128)
            stt_ = scat[slot]
            scs = []
            for h in range(2):
                a0 = b_lo + 8 * h
                sc = nc.gpsimd.local_scatter(
                    stt_[:, h * 1024:(h + 1) * 1024],
                    sdata[:, 3 * a0:3 * (a0 + 8)],
                    sidx[:, 3 * a0:3 * (a0 + 8)],
                    channels=128, num_elems=1024, num_idxs=24,
                )
                add_dep_helper(sc.ins, ll2.ins, reason="scatter after lib")
                add_dep_helper(
                    sc.ins, sdata_ops[s_i].ins, reason="scatter after sdata")
                add_dep_helper(
                    sc.ins, load_instrs["SIDX"].ins, reason="scatter after idx")
                if slot in scat_prev:
                    add_dep_helper(
                        sc.ins, scat_prev[slot].ins, reason="scatter buf WAR")
                scs.append(sc)
            eng = nc.sync if s_i % 2 == 0 else nc.scalar
            dma = eng.dma_start(OUT_p.ap()[:, csl], stt_[:])
            for sc in scs:
                add_dep_helper(dma.ins, sc.ins, reason="dma after scatter")
            scat_prev[slot] = dma

        # ---- 6b. DVE pattern supertiles ------------------------------------
        def mask_super(s_i):
            b_lo = s_i * SUPER
            b_hi = min(NB, b_lo + SUPER)
            nblk = b_hi - b_lo
            csl = slice(b_lo * 128, b_hi * 128)
            psl = slice((b_lo - NBS) * 128, (b_hi - NBS) * 128)
            pc = pc_pref[s_i]
            odt = fp8 if s_i >= FP8S else bf16
            stt_ = sup.tile([128, SUPER * 128], odt, tag="super")
            # PAT and the output region are COLUMN-major per supertile
            # ([p, c, b]); sv then broadcasts over the MIDDLE dim with its
            # own packed last dim, so one all-bf16 TT runs in the 0.5x mode.
            # The host unshards the transposed region.
            svv = (sv[:, b_lo:b_hi].unsqueeze(1)
                   .to_broadcast([128, 128, nblk]))
            nc.vector.tensor_tensor(
                stt_[:, :nblk * 128].rearrange("p (c b) -> p c b", b=nblk),
                pc[:, :nblk * 128].rearrange("p (c b) -> p c b", b=nblk),
                svv, mul,
            )
            if s_i >= NSUP - 2:
                eng2 = nc.gpsimd   # Pool ring is idle by the time these run
            else:
                eng2 = nc.scalar if s_i % 2 == 0 else nc.sync
            if s_i >= FP8S:
                csl2 = slice((b_lo - FP8S * SUPER) * 128,
                             (b_hi - FP8S * SUPER) * 128)
                eng2.dma_start(OUT2_p.ap()[:, csl2], stt_[:, :nblk * 128])
            else:
                eng2.dma_start(OUT_p.ap()[:, csl], stt_[:, :nblk * 128])

        # two more pat prefetches ride the SP/Act rings during the gather
        # window (dep on the broadcast loads keeps them out of the prefix)
        for j, s_i in enumerate(range(SPLIT + 4, SPLIT + 5)):
            b_lo, b_hi = s_i * SUPER, min(NB, (s_i + 1) * SUPER)
            psl = slice((b_lo - NBS) * 128, (b_hi - NBS) * 128)
            pc = pat.tile([128, SUPER * 128], bf16, name=f"pcg{s_i}", tag="pat")
            eng = nc.sync if j % 2 == 0 else nc.scalar
            pcd = eng.dma_start(pc[:, :(b_hi - b_lo) * 128], PAT_p.ap()[:, psl])
            for bl in bcast_loads:
                add_dep_helper(pcd.ins, bl.ins, reason="pat after bcast")
            pc_pref[s_i] = pc

        # remaining pattern supertiles overwrite the (dead after the gather)
        # broadcast table: bitcast bf16 views of bvtab, loaded while the
        # SP/Act rings are otherwise idle right after the gather
        for k, s_i in enumerate(range(SPLIT + 5, NSUP)):
            b_lo, b_hi = s_i * SUPER, min(NB, (s_i + 1) * SUPER)
            psl = slice((b_lo - NBS) * 128, (b_hi - NBS) * 128)
            pcv = bvtab[:, k * 1024:(k + 1) * 1024].bitcast(bf16)
            eng = nc.sync if s_i % 2 == 0 else nc.scalar
            pcd = eng.dma_start(pcv[:, :(b_hi - b_lo) * 128], PAT_p.ap()[:, psl])
            add_dep_helper(pcd.ins, gi.ins, reason="pat overwrites table WAR")
            pc_pref[s_i] = pcv

        # ---- schedule -------------------------------------------------------
        # DVE sums chunk A1 itself (its first post-gather work), so Pool's
        # queue is just [sums A2/B/C/D, lib, scatters] and the scatter
        # stream starts as soon as DVE posts sdata A1.
        sum_a1 = sum_chunk(*CH_A1, nc.vector)
        sum_a2 = sum_chunk(*CH_A2, nc.gpsimd)
        sum_b = sum_chunk(*CH_B, nc.gpsimd)
        sum_c = sum_chunk(*CH_C, nc.gpsimd)
        sum_d = sum_chunk(*CH_D, nc.gpsimd)
        ll2 = nc.gpsimd.load_library(library_config.local_scatter)
        add_dep_helper(ll2.ins, sum_d.ins, reason="lib switch after sums")

        scat_i = 0

        def emit_scatters(n):
            nonlocal scat_i
            for _ in range(n):
                if scat_i >= len(SCAT_ORDER):
                    return
                s_i = SCAT_ORDER[scat_i]
                scatter_super(s_i, scat_i % 4, ll2)
                scat_i += 1

        for (lo, hi), has_pair, sum_op in (
            (CH_A1, False, sum_a1), (CH_A2, False, sum_a2),
            (CH_B, True, sum_b),
        ):
            dve_compact_chunk(lo, hi, has_pair, sum_op)
            op = sdata_chunk(lo, hi)
            for s_i in range(lo // SUPER, hi // SUPER):
                sdata_ops[s_i] = op
            emit_scatters((hi - lo) // SUPER)
        dve_compact_chunk(*CH_C, False, sum_c)
        for s_i in range(SPLIT, (CH_C[1] + SUPER - 1) // SUPER):
            mask_super(s_i)
        dve_compact_chunk(*CH_D, False, sum_d)
        for s_i in range(17, NSUP):
            mask_super(s_i)
        emit_scatters(len(SCAT_ORDER))

    nc.compile()
    return nc


_PROGRAM = None
_TABLES = None


def _get_program():
    global _PROGRAM, _TABLES
    if _PROGRAM is None:
        _TABLES = _host_tables()
        _PROGRAM = _build_program()
    return _PROGRAM, _TABLES


def _feeds(core, W, per_core, ident, m16):
    t = per_core[core]
    wt = np.ascontiguousarray(
        W.T.reshape(4, 128, 128).transpose(1, 0, 2).reshape(128, 512)
    )
    sq = (W * W).sum(axis=1).astype(np.float32)
    return {
        "WT": wt, "WTM2": np.ascontiguousarray(-2.0 * wt),
        "SQP": np.ascontiguousarray(sq.reshape(128, 1)),
        "SQB": np.ascontiguousarray(np.broadcast_to(sq, (128, 128))),
        "M16": m16,
        "PAT": t["PAT"], "CM": t["CM"], "VSELM": t["VSELM"],
        "IXALL": t["IXALL"], "SIDX": t["SIDX"],
    }


def _unshard(out_pm: np.ndarray, out_fp8: np.ndarray) -> np.ndarray:
    """partition-major device shards -> [RC, 128] f32.

    Scatter supertiles (blocks < NBS) are [p, b, c]; pattern supertiles are
    column-major [p, c, b] per supertile; supertiles >= FP8S come from the
    fp8 shard.
    """
    out_pm = out_pm.astype(np.float32)
    out_fp8 = out_fp8.astype(np.float32)
    rows = np.empty((NB * 128, 128), np.float32)
    rows[:NBS * 128] = (
        out_pm[:, :NBS * 128].reshape(128, NBS, 128)
        .transpose(1, 0, 2).reshape(NBS * 128, 128)
    )
    for s_i in range(SPLIT, NSUP):
        b_lo = s_i * SUPER
        b_hi = min(NB, b_lo + SUPER)
        nblk = b_hi - b_lo
        if s_i >= FP8S:
            c0 = (b_lo - FP8S * SUPER) * 128
            seg = out_fp8[:, c0:c0 + nblk * 128].reshape(128, 128, nblk)
        else:
            seg = out_pm[:, b_lo * 128:b_hi * 128].reshape(128, 128, nblk)
        rows[b_lo * 128:b_hi * 128] = (
            seg.transpose(2, 0, 1).reshape(nblk * 128, 128)
        )
    return rows[:RC]


def kernel(W: np.ndarray) -> np.ndarray:
    nc, (per_core, ident, m16) = _get_program()
    W = np.ascontiguousarray(np.asarray(W, dtype=np.float32))
    in_maps = [_feeds(c, W, per_core, ident, m16) for c in range(NCORES)]
    res = run_bass_kernel_spmd(nc, in_maps, list(range(NCORES)))
    shards = [
        _unshard(np.asarray(res.results[c]["out"]),
                 np.asarray(res.results[c]["out2"]))
        for c in range(NCORES)
    ]
    return np.concatenate(shards, axis=0)
